# revision 25
# baseline (speedup 1.0000x reference)
"""Trainium2 Bass kernel for nn_PositionalEncoding (ragged positional-embedding gather).

Problem: routing [16, 4096] of 0/1 tokens. Node tokens (routing==0) and edge
tokens (routing==1) are compacted to the front (first 1024 nodes / 2048 edges),
each node position repeated 2x and each edge position 5x, then used to gather
rows of pos_embed [4096, 512] f32:
    node_pe [16, 2048, 512], edge_pe [16, 10240, 512]  (~384MB out, memory-bound)

Strategy (8 NeuronCores, data-parallel over batch, 2 batches/core), raw bass:
  - Host computes, per batch, the compacted node/edge position lists (first
    MAXN zeros / MAXE ones of the routing row, padded with 0 so every index
    stays valid), laid out in dma_gather's wrapped-index format: index i at
    partition i%16, column i//16, replicated across the 8 GPSIMD cores.
  - gpsimd `dma_gather` (InstDMAGatherAnt) pulls the unique compacted rows
    straight from pos_embed in HBM into SBUF: gathered slot i lands at
    [partition i%128, slot i//128]. Only ~6MB/batch is read instead of the
    full expanded 24MB.
  - The 2x/5x token repeats are position-independent, so they become STATIC
    strided HWDGE writes: copy k of unit j goes to output row j*REP+k, i.e.
    one [P, G, D] -> strided-DRAM DMA per copy k.
  - Tail rows beyond the actual token counts (reference pads with -1 ->
    clamped to pos_embed[0]) are filled on the host.

HBM traffic per core ~ 12MB gather reads + 48MB writes ~= close to the
~360GB/s memory roofline (~134us writes + ~34us reads, overlapped).
"""

import numpy as np

import concourse.bacc as bacc
import concourse.bass as bass
import concourse.mybir as mybir
from concourse.bass_utils import run_bass_kernel_spmd
from concourse.library_config import mlp as _mlp_lib

B, T, D = 16, 4096, 512
N_CORES = 8
BPC = B // N_CORES  # batches per core
MAXN, MAXE = 1024, 2048
NREP, EREP = 2, 5
P = 128
NG = MAXN // P   # gathered node slots per partition (8)
EG = MAXE // P   # gathered edge slots per partition (16)
NC16 = MAXN // 16  # node idx columns in wrapped layout (64)
EC16 = MAXE // 16  # edge idx columns (128)

_CACHE = {}


def _ensure_ntff_hook():
    """bass_utils' axon trace path imports antenv.axon_hooks, which this image
    lacks. Inject a shim wired to the ctypes NTFF hook from trn_agent_boot (or
    None, which makes bass_utils skip tracing gracefully)."""
    try:
        import antenv.axon_hooks  # noqa: F401

        return
    except ImportError:
        pass
    import sys
    import types

    hook = None
    try:
        from trn_agent_boot.trn_boot import _ntff_profile_via_ctypes

        hook = _ntff_profile_via_ctypes("/opt/axon/libaxon_pjrt.so")
    except Exception:
        hook = None
    m = types.ModuleType("antenv.axon_hooks")
    m.get_axon_ntff_profile_hook = lambda: hook
    m.set_axon_ntff_profile_hook = lambda h: None
    sys.modules["antenv.axon_hooks"] = m
    try:
        import antenv

        antenv.axon_hooks = m
    except ImportError:
        pass


def _patch_upload_artifacts():
    """The trace post-processing uploads artifacts to a share that may not be
    reachable here; fall back to the local path on failure."""
    import concourse.bass_utils as bu

    if getattr(bu.upload_artifacts, "_safe_wrapped", False):
        return
    orig = bu.upload_artifacts

    def safe_upload(tmpdir):
        try:
            return orig(tmpdir)
        except Exception:
            return str(tmpdir)

    safe_upload._safe_wrapped = True
    bu.upload_artifacts = safe_upload


def _build():
    """Build the SPMD Bass program (identical on all 8 cores), in raw bacc."""
    nc = bacc.Bacc("TRN2", num_swdge_queues=2)
    f32 = mybir.dt.float32
    i16 = mybir.dt.int16

    table = nc.declare_dram_parameter("table", [T, D], f32, isOutput=False)
    # wrapped gather indices, one combined load:
    # columns [nidx(b0) | eidx(b0) | nidx(b1) | eidx(b1)]
    idx = nc.declare_dram_parameter(
        "idx", [P, BPC * (NC16 + EC16)], i16, isOutput=False
    )
    outs = {}
    for b in range(BPC):
        # flat row-major row index = (g*P + p)*REP + k  == output token row
        outs[f"node{b}"] = nc.declare_dram_parameter(
            f"node{b}", [NG, P, NREP, D], f32, isOutput=True
        )
        outs[f"edge{b}"] = nc.declare_dram_parameter(
            f"edge{b}", [EG, P, EREP, D], f32, isOutput=True
        )

    # one dma_gather must stay <= 1024 indices (descriptor-carveout limit,
    # found empirically: 2048-idx gathers crash NRT), so gather in chunks.
    # 512 keeps the write pipelines fed early (first chunk ready sooner).
    CH = 512             # indices per gather chunk
    CHG = CH // P        # dst slots per chunk (4)
    CHC = CH // 16       # idx columns per chunk (32)
    n_chunks = {0: MAXN // CH, 1: MAXE // CH}  # node: 2, edge: 4

    import contextlib

    with contextlib.ExitStack() as stack:
        block = stack.enter_context(nc.Block())
        ld = stack.enter_context(nc.semaphore("ld"))
        ws = stack.enter_context(nc.semaphore("ws"))
        itile = stack.enter_context(
            nc.sbuf_tensor("itile", [P, BPC * (NC16 + EC16)], i16)
        )
        gn = [
            stack.enter_context(nc.sbuf_tensor(f"gn{b}", [P, NG, D], f32))
            for b in range(BPC)
        ]
        ge = [
            stack.enter_context(nc.sbuf_tensor(f"ge{b}", [P, EG, D], f32))
            for b in range(BPC)
        ]
        # one semaphore per gather chunk (the race detector rejects unordered
        # multi-updates of one sem by DMAGatherAnt instructions)
        gsems = {}
        for b in range(BPC):
            for t in range(2):
                for c in range(n_chunks[t]):
                    gsems[b, t, c] = stack.enter_context(
                        nc.semaphore(f"g{b}_{t}_{c}")
                    )

        # chunk stream in gather / write-consumption order
        chunks = []
        for b in range(BPC):
            for t, tiles, name, rep in (
                (0, gn, f"node{b}", NREP),
                (1, ge, f"edge{b}", EREP),
            ):
                for c in range(n_chunks[t]):
                    chunks.append((b, t, c, tiles, name, rep))
        n_w_total = sum(rep for (_, _, _, _, _, rep) in chunks)

        @block.gpsimd
        def _(g: bass.BassGpSimd):
            g.load_library(_mlp_lib)  # DMAGatherAnt lives in the mlp ucode lib
            g.wait_ge(ld, 16)
            col = 0
            for i, (b, t, c, tiles, _, _) in enumerate(chunks):
                g.dma_gather(
                    tiles[b][:, c * CHG : (c + 1) * CHG, :],
                    table[:],
                    itile[:, col : col + CHC],
                    CH,
                    CH,
                    D,
                    queue_num=i % 2,  # parallel SWDGE descriptor generation
                ).then_inc(gsems[b, t, c], 16)
                col += CHC

        def write_stream(eng, my_chunks, final_wait):
            with nc.allow_non_contiguous_dma(reason="strided repeat-expand writes"):
                for b, t, c, tiles, name, rep in my_chunks:
                    eng.wait_ge(gsems[b, t, c], 16)
                    sl = slice(c * CHG, (c + 1) * CHG)
                    for k in range(rep):
                        # out rows (g*P+p)*rep + k <- gathered [p, g, :]
                        eng.dma_start(
                            out=outs[name][sl, :, k, :].transpose([1, 0, 2]),
                            in_=tiles[b][:, sl, :],
                        ).then_inc(ws, 16)
            if final_wait:
                eng.wait_ge(ws, n_w_total * 16)

        @block.sync
        def _(sync: bass.BassEngine):
            sync.dma_start(out=itile[:], in_=idx[:]).then_inc(ld, 16)
            write_stream(sync, chunks[0::2], final_wait=True)

        @block.scalar
        def _(act: bass.BassEngine):
            write_stream(act, chunks[1::2], final_wait=False)

    return nc


def get_nc():
    if "nc" not in _CACHE:
        nc = _build()
        nc.finalize()  # Bacc: runs compile() (reg alloc, library-load insert)
        _CACHE["nc"] = nc
    return _CACHE["nc"]


def _wrap16(arr, ncols):
    """dma_gather index layout: index i at [i%16, i//16], replicated to 128
    partitions (8 copies for the 8 GPSIMD cores)."""
    w = arr.reshape(ncols, 16).T  # [16, ncols]
    return np.tile(w, (8, 1))     # [128, ncols]


def host_indices(routing):
    """Compacted node/edge position lists per batch, padded with 0 (a valid
    index; those tail slots are overwritten by the host fill), in wrapped
    int16 layout. Also the clipped per-batch counts."""
    r = np.asarray(routing)
    nidx = np.zeros((B, P, NC16), dtype=np.int16)
    eidx = np.zeros((B, P, EC16), dtype=np.int16)
    counts0 = np.empty(B, dtype=np.int64)
    counts1 = np.empty(B, dtype=np.int64)
    for b in range(B):
        z = np.nonzero(r[b] == 0)[0]
        o = np.nonzero(r[b] != 0)[0]
        counts0[b] = min(len(z), MAXN)
        counts1[b] = min(len(o), MAXE)
        zp = np.zeros(MAXN, dtype=np.int16)
        zp[: counts0[b]] = z[: counts0[b]]
        op = np.zeros(MAXE, dtype=np.int16)
        op[: counts1[b]] = o[: counts1[b]]
        nidx[b] = _wrap16(zp, NC16)
        eidx[b] = _wrap16(op, EC16)
    return nidx, eidx, counts0, counts1


def kernel(routing, max_nodes, max_edges, pos_embed):
    assert int(max_nodes) == MAXN and int(max_edges) == MAXE
    routing = np.asarray(routing, dtype=np.int32)
    pos_embed = np.ascontiguousarray(np.asarray(pos_embed, dtype=np.float32))
    assert routing.shape == (B, T) and pos_embed.shape == (T, D)

    nidx, eidx, counts0, counts1 = host_indices(routing)

    in_maps = []
    for core in range(N_CORES):
        lo = core * BPC
        idx_core = np.concatenate(
            sum([[nidx[lo + b], eidx[lo + b]] for b in range(BPC)], []), axis=1
        )
        in_maps.append({"table": pos_embed, "idx": idx_core})

    nc = get_nc()
    _ensure_ntff_hook()
    _patch_upload_artifacts()
    bkr = run_bass_kernel_spmd(nc, in_maps, list(range(N_CORES)))
    _CACHE["last_bkr"] = bkr  # test harness introspection (exec_time_ns etc.)
    res = bkr.results

    node_pe = np.empty((B, MAXN * NREP, D), dtype=np.float32)
    edge_pe = np.empty((B, MAXE * EREP, D), dtype=np.float32)
    for core in range(N_CORES):
        for b in range(BPC):
            bi = core * BPC + b
            node_pe[bi] = res[core][f"node{b}"].reshape(MAXN * NREP, D)
            edge_pe[bi] = res[core][f"edge{b}"].reshape(MAXE * EREP, D)
            # Tail rows beyond the actual token counts: reference pads the
            # compacted position lists with -1 -> clamped to pos_embed[0].
            node_pe[bi, NREP * int(counts0[bi]) :] = pos_embed[0]
            edge_pe[bi, EREP * int(counts1[bi]) :] = pos_embed[0]
    return node_pe, edge_pe


# revision 28
# speedup vs baseline: 1.1129x; 1.1129x over previous
"""Trainium2 Bass kernel for nn_PositionalEncoding (ragged positional-embedding gather).

Problem: routing [16, 4096] of 0/1 tokens. Node tokens (routing==0) and edge
tokens (routing==1) are compacted to the front (first 1024 nodes / 2048 edges),
each node position repeated 2x and each edge position 5x, then used to gather
rows of pos_embed [4096, 512] f32:
    node_pe [16, 2048, 512], edge_pe [16, 10240, 512]  (~384MB out, memory-bound)

Strategy (8 NeuronCores, data-parallel over batch, 2 batches/core), raw bass:
  - Host computes, per batch, the compacted node/edge position lists (first
    MAXN zeros / MAXE ones of the routing row, padded with 0 so every index
    stays valid), laid out in dma_gather's wrapped-index format: index i at
    partition i%16, column i//16, replicated across the 8 GPSIMD cores.
  - gpsimd `dma_gather` (InstDMAGatherAnt) pulls the unique compacted rows
    straight from pos_embed in HBM into SBUF: gathered slot i lands at
    [partition i%128, slot i//128]. Only ~6MB/batch is read instead of the
    full expanded 24MB.
  - The 2x/5x token repeats are position-independent, so they become STATIC
    strided HWDGE writes: copy k of unit j goes to output row j*REP+k, i.e.
    one [P, G, D] -> strided-DRAM DMA per copy k.
  - Tail rows beyond the actual token counts (reference pads with -1 ->
    clamped to pos_embed[0]) are filled on the host.

HBM traffic per core ~ 12MB gather reads + 48MB writes ~= close to the
~360GB/s memory roofline (~134us writes + ~34us reads, overlapped).
"""

import numpy as np

import concourse.bacc as bacc
import concourse.bass as bass
import concourse.mybir as mybir
from concourse.bass_utils import run_bass_kernel_spmd
from concourse.library_config import mlp as _mlp_lib

B, T, D = 16, 4096, 512
N_CORES = 8
BPC = B // N_CORES  # batches per core
MAXN, MAXE = 1024, 2048
NREP, EREP = 2, 5
P = 128
NG = MAXN // P   # gathered node slots per partition (8)
EG = MAXE // P   # gathered edge slots per partition (16)
NC16 = MAXN // 16  # node idx columns in wrapped layout (64)
EC16 = MAXE // 16  # edge idx columns (128)

_CACHE = {}


def _ensure_ntff_hook():
    """bass_utils' axon trace path imports antenv.axon_hooks, which this image
    lacks. Inject a shim wired to the ctypes NTFF hook from trn_agent_boot (or
    None, which makes bass_utils skip tracing gracefully)."""
    try:
        import antenv.axon_hooks  # noqa: F401

        return
    except ImportError:
        pass
    import sys
    import types

    hook = None
    try:
        from trn_agent_boot.trn_boot import _ntff_profile_via_ctypes

        hook = _ntff_profile_via_ctypes("/opt/axon/libaxon_pjrt.so")
    except Exception:
        hook = None
    m = types.ModuleType("antenv.axon_hooks")
    m.get_axon_ntff_profile_hook = lambda: hook
    m.set_axon_ntff_profile_hook = lambda h: None
    sys.modules["antenv.axon_hooks"] = m
    try:
        import antenv

        antenv.axon_hooks = m
    except ImportError:
        pass


def _patch_upload_artifacts():
    """The trace post-processing uploads artifacts to a share that may not be
    reachable here; fall back to the local path on failure."""
    import concourse.bass_utils as bu

    if getattr(bu.upload_artifacts, "_safe_wrapped", False):
        return
    orig = bu.upload_artifacts

    def safe_upload(tmpdir):
        try:
            return orig(tmpdir)
        except Exception:
            return str(tmpdir)

    safe_upload._safe_wrapped = True
    bu.upload_artifacts = safe_upload


def _build():
    """Build the SPMD Bass program (identical on all 8 cores), in raw bacc."""
    nc = bacc.Bacc("TRN2", num_swdge_queues=2)
    f32 = mybir.dt.float32
    i16 = mybir.dt.int16

    table = nc.declare_dram_parameter("table", [T, D], f32, isOutput=False)
    # wrapped gather indices, one combined load:
    # columns [nidx(b0) | eidx(b0) | nidx(b1) | eidx(b1)]
    idx = nc.declare_dram_parameter(
        "idx", [P, BPC * (NC16 + EC16)], i16, isOutput=False
    )
    outs = {}
    for b in range(BPC):
        # flat row-major row index = (g*P + p)*REP + k  == output token row
        outs[f"node{b}"] = nc.declare_dram_parameter(
            f"node{b}", [NG, P, NREP, D], f32, isOutput=True
        )
        outs[f"edge{b}"] = nc.declare_dram_parameter(
            f"edge{b}", [EG, P, EREP, D], f32, isOutput=True
        )

    # one dma_gather must stay <= 1024 indices (descriptor-carveout limit,
    # found empirically: 2048-idx gathers crash NRT), so gather in chunks.
    # First batch's node gather is split small so the write pipelines start
    # early; everything else uses 1024 (fewer instructions = less serial gap).
    chunk_sizes = {
        (0, 0): [512, 512],
        (0, 1): [1024, 1024],
        (1, 0): [1024],
        (1, 1): [1024, 1024],
    }  # [(b, t)] -> chunk index counts

    import contextlib

    with contextlib.ExitStack() as stack:
        block = stack.enter_context(nc.Block())
        ld = stack.enter_context(nc.semaphore("ld"))
        ws = stack.enter_context(nc.semaphore("ws"))
        itile = stack.enter_context(
            nc.sbuf_tensor("itile", [P, BPC * (NC16 + EC16)], i16)
        )
        gn = [
            stack.enter_context(nc.sbuf_tensor(f"gn{b}", [P, NG, D], f32))
            for b in range(BPC)
        ]
        ge = [
            stack.enter_context(nc.sbuf_tensor(f"ge{b}", [P, EG, D], f32))
            for b in range(BPC)
        ]
        # chunk descriptors: (b, t, c, unit_start, n_units, tile, name, rep)
        # built in gather / write-consumption order
        chunks = []
        col_of = {}
        col = 0
        for b in range(BPC):
            for t, tiles, name, rep in (
                (0, gn, f"node{b}", NREP),
                (1, ge, f"edge{b}", EREP),
            ):
                u0 = 0
                for c, ch in enumerate(chunk_sizes[b, t]):
                    chunks.append((b, t, c, u0, ch, tiles[b], name, rep))
                    col_of[b, t, c] = col
                    col += ch // 16
                    u0 += ch
        n_w_total = sum(ck[7] for ck in chunks)  # rep per chunk

        # one semaphore per gather chunk (the race detector rejects unordered
        # multi-updates of one sem by DMAGatherAnt instructions)
        gsems = {
            (b, t, c): stack.enter_context(nc.semaphore(f"g{b}_{t}_{c}"))
            for (b, t, c, *_r) in chunks
        }

        @block.gpsimd
        def _(g: bass.BassGpSimd):
            g.load_library(_mlp_lib)  # DMAGatherAnt lives in the mlp ucode lib
            g.wait_ge(ld, 16)
            for i, (b, t, c, u0, ch, tile_, _, _) in enumerate(chunks):
                cc = col_of[b, t, c]
                g.dma_gather(
                    tile_[:, u0 // P : (u0 + ch) // P, :],
                    table[:],
                    itile[:, cc : cc + ch // 16],
                    ch,
                    ch,
                    D,
                    queue_num=i % 2,  # parallel SWDGE descriptor generation
                ).then_inc(gsems[b, t, c], 16)

        def write_stream(eng, my_chunks, final_wait):
            with nc.allow_non_contiguous_dma(reason="strided repeat-expand writes"):
                for b, t, c, u0, ch, tile_, name, rep in my_chunks:
                    eng.wait_ge(gsems[b, t, c], 16)
                    sl = slice(u0 // P, (u0 + ch) // P)
                    for k in range(rep):
                        # out rows (g*P+p)*rep + k <- gathered [p, g, :]
                        eng.dma_start(
                            out=outs[name][sl, :, k, :].transpose([1, 0, 2]),
                            in_=tile_[:, sl, :],
                        ).then_inc(ws, 16)
            if final_wait:
                eng.wait_ge(ws, n_w_total * 16)

        by_key = {(b, t, c): ck for ck in chunks for (b, t, c) in [ck[:3]]}
        sp_chunks = [
            by_key[0, 0, 0], by_key[0, 0, 1], by_key[1, 1, 0], by_key[1, 1, 1]
        ]
        act_chunks = [by_key[0, 1, 0], by_key[0, 1, 1], by_key[1, 0, 0]]

        @block.sync
        def _(sync: bass.BassEngine):
            sync.dma_start(out=itile[:], in_=idx[:]).then_inc(ld, 16)
            write_stream(sync, sp_chunks, final_wait=True)

        @block.scalar
        def _(act: bass.BassEngine):
            write_stream(act, act_chunks, final_wait=False)

    return nc


def get_nc():
    if "nc" not in _CACHE:
        nc = _build()
        nc.finalize()  # Bacc: runs compile() (reg alloc, library-load insert)
        _CACHE["nc"] = nc
    return _CACHE["nc"]


def _wrap16(arr, ncols):
    """dma_gather index layout: index i at [i%16, i//16], replicated to 128
    partitions (8 copies for the 8 GPSIMD cores)."""
    w = arr.reshape(ncols, 16).T  # [16, ncols]
    return np.tile(w, (8, 1))     # [128, ncols]


def host_indices(routing):
    """Compacted node/edge position lists per batch, padded with 0 (a valid
    index; those tail slots are overwritten by the host fill), in wrapped
    int16 layout. Also the clipped per-batch counts."""
    r = np.asarray(routing)
    nidx = np.zeros((B, P, NC16), dtype=np.int16)
    eidx = np.zeros((B, P, EC16), dtype=np.int16)
    counts0 = np.empty(B, dtype=np.int64)
    counts1 = np.empty(B, dtype=np.int64)
    for b in range(B):
        z = np.nonzero(r[b] == 0)[0]
        o = np.nonzero(r[b] != 0)[0]
        counts0[b] = min(len(z), MAXN)
        counts1[b] = min(len(o), MAXE)
        zp = np.zeros(MAXN, dtype=np.int16)
        zp[: counts0[b]] = z[: counts0[b]]
        op = np.zeros(MAXE, dtype=np.int16)
        op[: counts1[b]] = o[: counts1[b]]
        nidx[b] = _wrap16(zp, NC16)
        eidx[b] = _wrap16(op, EC16)
    return nidx, eidx, counts0, counts1


def kernel(routing, max_nodes, max_edges, pos_embed):
    assert int(max_nodes) == MAXN and int(max_edges) == MAXE
    routing = np.asarray(routing, dtype=np.int32)
    pos_embed = np.ascontiguousarray(np.asarray(pos_embed, dtype=np.float32))
    assert routing.shape == (B, T) and pos_embed.shape == (T, D)

    nidx, eidx, counts0, counts1 = host_indices(routing)

    in_maps = []
    for core in range(N_CORES):
        lo = core * BPC
        idx_core = np.concatenate(
            sum([[nidx[lo + b], eidx[lo + b]] for b in range(BPC)], []), axis=1
        )
        in_maps.append({"table": pos_embed, "idx": idx_core})

    nc = get_nc()
    _ensure_ntff_hook()
    _patch_upload_artifacts()
    bkr = run_bass_kernel_spmd(nc, in_maps, list(range(N_CORES)))
    _CACHE["last_bkr"] = bkr  # test harness introspection (exec_time_ns etc.)
    res = bkr.results

    node_pe = np.empty((B, MAXN * NREP, D), dtype=np.float32)
    edge_pe = np.empty((B, MAXE * EREP, D), dtype=np.float32)
    for core in range(N_CORES):
        for b in range(BPC):
            bi = core * BPC + b
            node_pe[bi] = res[core][f"node{b}"].reshape(MAXN * NREP, D)
            edge_pe[bi] = res[core][f"edge{b}"].reshape(MAXE * EREP, D)
            # Tail rows beyond the actual token counts: reference pads the
            # compacted position lists with -1 -> clamped to pos_embed[0].
            node_pe[bi, NREP * int(counts0[bi]) :] = pos_embed[0]
            edge_pe[bi, EREP * int(counts1[bi]) :] = pos_embed[0]
    return node_pe, edge_pe
